# revision 22
# baseline (speedup 1.0000x reference)
"""Trainium2 Bass kernel for nn_Attention (dense transformer block).

Reference computation per batch image (B=8, H=W=64, C=192, D=24, L=4096):
    q = x @ w_q; k = x @ w_k; v = x @ w_v          # [L, D]
    s = q @ k^T                                    # [L, L]
    beta = softmax(s, axis=-1)
    out = gamma * (beta @ v) @ w_o + x             # [L, C]

Sharding: pure data parallel, one image per NeuronCore (8 cores).

Per-core dataflow (matmuls bf16, fp32 PSUM accumulate). The PE array is
packed 4x both ways since the head dim (24) and value dim (25) waste the
128x128 array:
  - scores are row-tiled: 4 key chunks computed concurrently in row groups
    (32g..32g+23) against group-stacked q^T/k^T ([121, L] tiles built by
    one matmul with host-side stacked weights wq_stack [C, 121]).
  - attention accumulation is col-tiled: vhat chunks (M=32: v | ones | 0pad)
    write group 32g of the stacked partials [128, W] via tile_position.
  - exp on ScalarE processes a whole quad tile [128, 4*W] in one ACTIVATE.
  - epilogue: un = partials_T @ wo_stack [128, 193] (group-replicated
    gamma*w_o; col 192 collects the 4 softmax-denominator rows), then
    out = un[:, :192]/un[:, 192] + x on DVE.

Softmax w/o max subtraction is safe: scores range ~[-50, 54];
exp(54) ~ 2e23 << fp32/bf16 max; row sums < 1e27.
"""

import numpy as np

import concourse.bass as bass
import concourse.tile as tile
from concourse import bacc, mybir
from concourse.bass_utils import run_bass_kernel_spmd
from concourse.masks import make_identity

F32 = mybir.dt.float32
BF16 = mybir.dt.bfloat16

B = 8
L = 4096          # tokens per image (64*64)
C = 192           # channels
D = 24            # head dim (q/k/v)
G = 4             # PE array packing groups
NCH = L // 128    # 32 chunks of 128 tokens
W = 512           # i-window (moving free dim per matmul)
NIW = L // W      # 16 i-windows
WIN = 512         # projection window (rhs free dim)
NWIN = L // WIN   # 8 windows
NQ = NCH // G     # 8 quads of key chunks
SROWS = 32 * (G - 1) + D      # 120 used rows of stacked q/k tiles
VW = 32           # padded vhat chunk width (v | ones | zeros)


def build_graph():
    """Build the single-core Bass graph (SPMD: identical on all 8 cores)."""
    nc = bacc.Bacc(
        "TRN2", target_bir_lowering=False, debug=False, num_devices=8,
        name="attn_dp",
    )

    x_ext = nc.dram_tensor("x", [L, C], F32, kind="ExternalInput").ap()
    # group-stacked projection weights [C, 121]: col 32g+d = w[:, d]
    wqs_ext = nc.dram_tensor("wq_stack", [C, 128], F32,
                             kind="ExternalInput").ap()
    wks_ext = nc.dram_tensor("wk_stack", [C, 128], F32,
                             kind="ExternalInput").ap()
    wv_ext = nc.dram_tensor("wv", [C, D], F32, kind="ExternalInput").ap()
    # wo_stack [128, 193]: rows 32g+d = gamma * w_o[d]; rows 32g+24 col 192 = 1
    wos_ext = nc.dram_tensor("wo_stack", [128, C + 1], F32,
                             kind="ExternalInput").ap()
    out_ext = nc.dram_tensor("out", [L, C], F32, kind="ExternalOutput").ap()

    with tile.TileContext(nc) as tc:
        _build(tc, x_ext, wqs_ext, wks_ext, wv_ext, wos_ext, out_ext)

    nc.compile()
    return nc


def _build(tc, x_ext, wqs_ext, wks_ext, wv_ext, wos_ext, out_ext):
    nc = tc.nc

    with (
        # ---- persistent SBUF ----
        tc.tile_pool(name="const", bufs=1) as const_pool,
        tc.tile_pool(name="xsb", bufs=1) as x_pool,
        tc.tile_pool(name="xbf", bufs=4) as xbf_pool,
        tc.tile_pool(name="xT", bufs=1) as xT_pool,
        tc.tile_pool(name="qkT", bufs=1) as qkT_pool,
        tc.tile_pool(name="vhat", bufs=1) as vhat_pool,
        tc.tile_pool(name="expS", bufs=4) as expS_pool,
        tc.tile_pool(name="pt", bufs=2) as pt_pool,
        tc.tile_pool(name="outst", bufs=4) as outst_pool,
        tc.tile_pool(name="rden", bufs=4) as r_pool,
        # ---- PSUM (8 banks): scores 2x2 + partials 2x1 + epi 2x1 ----
        tc.tile_pool(name="ps_s", bufs=2, space="PSUM") as ps_scores,
        tc.tile_pool(name="ps_acc", bufs=2, space="PSUM") as ps_partials,
        tc.tile_pool(name="ps_e", bufs=2, space="PSUM") as ps_epi,
    ):
        # ================= constants =================
        ident = const_pool.tile([128, 128], BF16)
        make_identity(nc, ident[:])

        # PE warm-up: ~4.5us of dense back-to-back matmuls with no deps so the
        # HAM clock gate un-throttles (1.2 -> 2.4 GHz) before the main pipeline
        warm_rhs = const_pool.tile([128, 256], BF16)
        nc.gpsimd.memset(warm_rhs[:], 0.0)
        warm_ps = ps_epi.tile([128, 256], F32, tag="e")
        for _ in range(18):
            nc.tensor.matmul(warm_ps[:], ident[:], warm_rhs[:],
                             start=True, stop=True)

        # zeros for the partials-bank init matmul (K=1): out = zl.T @ zr = 0
        zl = const_pool.tile([1, 128], BF16)
        zr = const_pool.tile([1, W], BF16)
        nc.gpsimd.memset(zl[:], 0.0)
        nc.gpsimd.memset(zr[:], 0.0)

        wstage = const_pool.tile([128, 760], F32)  # fp32 staging for weights
        def load_weight_bf(ext, rows, cols, stage_off, tag):
            st = wstage[:rows, stage_off:stage_off + cols]
            nc.sync.dma_start(st, ext)
            t = const_pool.tile([rows, cols], BF16, tag=tag)
            nc.vector.tensor_copy(t[:], st)
            return t

        SR = 128
        wqsa = load_weight_bf(wqs_ext[0:128, :], 128, SR, 0, "wqsa")
        wqsb = load_weight_bf(wqs_ext[128:192, :], 64, SR, SR, "wqsb")
        wksa = load_weight_bf(wks_ext[0:128, :], 128, SR, 2 * SR, "wksa")
        wksb = load_weight_bf(wks_ext[128:192, :], 64, SR, 3 * SR, "wksb")
        wva = load_weight_bf(wv_ext[0:128, :], 128, D, 4 * SR, "wva")
        wvb = load_weight_bf(wv_ext[128:192, :], 64, D, 4 * SR + D, "wvb")
        wos = load_weight_bf(wos_ext, 128, C + 1, 4 * SR + 2 * D, "wos")

        # ================= x load + transpose =================
        x_sb = x_pool.tile([128, NCH * C], F32)       # chunk c at cols [C*c, ...)
        xTa = xT_pool.tile([128, L], BF16)            # x^T rows 0..127 (channels)
        xTb = xT_pool.tile([64, L], BF16)             # x^T rows 128..191
        kTs = qkT_pool.tile([128, L], BF16)           # stacked k^T replicas
        qTs = qkT_pool.tile([128, L], BF16)           # stacked q^T replicas
        vhat = vhat_pool.tile([128, NCH * VW], BF16)  # v | ones | zero pad
        nc.gpsimd.memset(vhat[:], 0.0)
        ones_view = vhat.rearrange("p (j d) -> p j d", d=VW)[:, :, D]
        nc.gpsimd.memset(ones_view, 1.0)

        for ch in range(NCH):
            nc.sync.dma_start(x_sb[:, C * ch:C * (ch + 1)],
                              x_ext[128 * ch:128 * (ch + 1), :])

        def transpose_window(w):
            """x^T slab for token window w (4 chunks of 128)."""
            psA = ps_scores.tile([128, WIN], BF16, tag="s", name=f"psA{w}")
            psB = ps_scores.tile([64, WIN], BF16, tag="s", name=f"psB{w}")
            for t in range(4):
                ch = 4 * w + t
                xbf = xbf_pool.tile([128, C], BF16, tag="xbf", name=f"xbf{ch}")
                nc.gpsimd.tensor_copy(xbf[:], x_sb[:, C * ch:C * (ch + 1)])
                nc.tensor.transpose(psA[:, 128 * t:128 * (t + 1)],
                                    xbf[:, 0:128], ident[:])
                nc.tensor.transpose(psB[:, 128 * t:128 * (t + 1)],
                                    xbf[:, 128:192], ident[:])
            nc.vector.tensor_copy(xTa[:, WIN * w:WIN * (w + 1)], psA[:])
            nc.vector.tensor_copy(xTb[:, WIN * w:WIN * (w + 1)], psB[:])

        def project(dst, wa, wb, w, nm):
            ps = ps_epi.tile([128, WIN], F32, tag="e", name=f"pj{nm}{w}")
            sl = slice(WIN * w, WIN * (w + 1))
            nc.tensor.matmul(ps[:], wa[:], xTa[:, sl], start=True, stop=False)
            nc.tensor.matmul(ps[:], wb[:], xTb[:, sl], start=False, stop=True)
            nc.vector.tensor_copy(dst[:, sl], ps[:])

        def vhat_quad(t):
            for j in range(4 * t, 4 * t + 4):
                ps = ps_epi.tile([128, D], F32, tag="e", name=f"pv{j}")
                jsl = slice(128 * j, 128 * (j + 1))
                nc.tensor.matmul(ps[:], xTa[:, jsl], wva[:],
                                 start=True, stop=False)
                nc.tensor.matmul(ps[:], xTb[:, jsl], wvb[:],
                                 start=False, stop=True)
                nc.vector.tensor_copy(vhat[:, VW * j:VW * j + D], ps[:])

        # prologue: only window 0 of everything; the rest is emitted
        # just-in-time inside the iw=0 loop and hides under the exp stream
        transpose_window(0)
        project(kTs, wksa, wksb, 0, "k")
        project(qTs, wqsa, wqsb, 0, "q")
        vhat_quad(0)

        # ================= main loop =================
        # i-windows of W=512; key chunks in quads of 4 (row groups 0..3).
        # Each row group's scores land in a distinct PSUM bank (HW rule):
        # groups 0/1 -> scA banks 0/1, groups 2/3 -> scB banks 0/1.
        for iw in range(NIW):
            isl = slice(W * iw, W * (iw + 1))
            if iw >= 1:
                project(qTs, wqsa, wqsb, iw, "q")
            partials = ps_partials.tile([128, W], F32)   # 4 groups of attn^T
            # zero-init the bank and set has_written on all 128 partitions so
            # the col-tiled accumulating matmuls below can all use start=False
            nc.tensor.matmul(partials[:, :], zl[:], zr[:],
                             start=True, stop=False, skip_group_check=True)
            for t in range(NQ):
                scs = [ps_scores.tile([128, 1024], F32, tag="s", name=f"sc{iw}_{t}_0"),
                       ps_scores.tile([128, 1024], F32, tag="s", name=f"sc{iw}_{t}_1")]
                for g in range(G):
                    j = G * t + g
                    nc.tensor.matmul(
                        scs[g // 2][:, 512 * (g % 2):512 * (g % 2 + 1)],
                        kTs[32 * g:32 * g + 32, 128 * j:128 * (j + 1)],
                        qTs[32 * g:32 * g + 32, isl],
                        start=True, stop=True,
                        tile_position=(32 * g, 0),
                    )
                ess = []
                for h in range(2):
                    es = expS_pool.tile([128, 1024], BF16, name=f"es{iw}_{t}_{h}", tag="es")
                    nc.scalar.activation(es[:], scs[h][:],
                                         mybir.ActivationFunctionType.Exp)
                    ess.append(es)
                if iw == 0 and t < NQ - 1:
                    # just-in-time prologue work for the next quad's chunks,
                    # scheduled into PE slack while ScalarE runs the exps
                    transpose_window(t + 1)
                    project(kTs, wksa, wksb, t + 1, "k")
                    vhat_quad(t + 1)
                for g in (2, 3, 0, 1):
                    j = G * t + g
                    nc.tensor.matmul(
                        partials[32 * g:32 * g + VW, :],
                        vhat[:, VW * j:VW * (j + 1)],
                        ess[g // 2][:, 512 * (g % 2):512 * (g % 2 + 1)],
                        start=False, stop=(t == NQ - 1),
                        tile_position=(0, 32 * g),
                        skip_group_check=True,
                    )

            # ---- i-window epilogue ----
            ptb = pt_pool.tile([128, W], BF16, name=f"ptb{iw}", tag="ptb")
            nc.vector.tensor_copy(ptb[:], partials[:])
            for s in range(W // 128):
                cidx = (W // 128) * iw + s  # 128-token chunk index
                ep = ps_epi.tile([128, C + 1], F32, tag="e", name=f"ep{iw}_{s}")
                nc.tensor.matmul(ep[:], ptb[:, 128 * s:128 * (s + 1)],
                                 wos[:], start=True, stop=True)
                rr = r_pool.tile([128, 1], F32, name=f"rr{iw}_{s}", tag="rr")
                nc.vector.reciprocal(rr[:], ep[:, C:C + 1])
                ot = outst_pool.tile([128, C], F32, name=f"ot{iw}_{s}", tag="ot")
                nc.vector.tensor_scalar_mul(ot[:], ep[:, 0:C], rr[:])
                nc.vector.tensor_add(ot[:], ot[:],
                                     x_sb[:, C * cidx:C * (cidx + 1)])
                nc.sync.dma_start(out_ext[128 * cidx:128 * (cidx + 1), :], ot[:])


_CACHE = {}


def _get_graph():
    if "nc" not in _CACHE:
        _CACHE["nc"] = build_graph()
    return _CACHE["nc"]


def make_in_maps(tensor, w_q, w_k, w_v, w_o, gamma):
    x = np.ascontiguousarray(np.asarray(tensor, dtype=np.float32)).reshape(B, L, C)
    wq = np.asarray(w_q, dtype=np.float32)
    wk = np.asarray(w_k, dtype=np.float32)
    wv = np.ascontiguousarray(np.asarray(w_v, dtype=np.float32))
    wo = np.asarray(w_o, dtype=np.float32)

    wq_stack = np.zeros((C, 128), dtype=np.float32)
    wk_stack = np.zeros((C, 128), dtype=np.float32)
    for g in range(G):
        wq_stack[:, 32 * g:32 * g + D] = wq
        wk_stack[:, 32 * g:32 * g + D] = wk

    wo_stack = np.zeros((128, C + 1), dtype=np.float32)
    for g in range(G):
        wo_stack[32 * g:32 * g + D, :C] = wo * np.float32(gamma)
        wo_stack[32 * g + D, C] = 1.0

    return [
        {"x": np.ascontiguousarray(x[b]), "wq_stack": wq_stack,
         "wk_stack": wk_stack, "wv": wv, "wo_stack": wo_stack}
        for b in range(B)
    ]


def kernel(tensor, w_q, w_k, w_v, w_o, gamma):
    nc = _get_graph()
    in_maps = make_in_maps(tensor, w_q, w_k, w_v, w_o, gamma)
    res = run_bass_kernel_spmd(nc, in_maps, core_ids=list(range(B)))
    out = np.stack([np.asarray(res.results[b]["out"]) for b in range(B)])
    return out.reshape(B, 64, 64, C).astype(np.float32)


# revision 23
# speedup vs baseline: 1.2203x; 1.2203x over previous
"""Trainium2 Bass kernel for nn_Attention (dense transformer block).

Reference computation per batch image (B=8, H=W=64, C=192, D=24, L=4096):
    q = x @ w_q; k = x @ w_k; v = x @ w_v          # [L, D]
    s = q @ k^T                                    # [L, L]
    beta = softmax(s, axis=-1)
    out = gamma * (beta @ v) @ w_o + x             # [L, C]

Sharding: pure data parallel, one image per NeuronCore (8 cores).

Per-core dataflow (matmuls bf16, fp32 PSUM accumulate). The PE array is
packed 4x both ways since the head dim (24) and value dim (25) waste the
128x128 array:
  - scores are row-tiled: 4 key chunks computed concurrently in row groups
    (32g..32g+23) against group-stacked q^T/k^T ([121, L] tiles built by
    one matmul with host-side stacked weights wq_stack [C, 121]).
  - attention accumulation is col-tiled: vhat chunks (M=32: v | ones | 0pad)
    write group 32g of the stacked partials [128, W] via tile_position.
  - exp on ScalarE processes a whole quad tile [128, 4*W] in one ACTIVATE.
  - epilogue: un = partials_T @ wo_stack [128, 193] (group-replicated
    gamma*w_o; col 192 collects the 4 softmax-denominator rows), then
    out = un[:, :192]/un[:, 192] + x on DVE.

Softmax w/o max subtraction is safe: scores range ~[-50, 54];
exp(54) ~ 2e23 << fp32/bf16 max; row sums < 1e27.
"""

import numpy as np

import concourse.bass as bass
import concourse.tile as tile
from concourse import bacc, mybir
from concourse.bass_utils import run_bass_kernel_spmd
from concourse.masks import make_identity

F32 = mybir.dt.float32
BF16 = mybir.dt.bfloat16

B = 8
L = 4096          # tokens per image (64*64)
C = 192           # channels
D = 24            # head dim (q/k/v)
G = 4             # PE array packing groups
NCH = L // 128    # 32 chunks of 128 tokens
W = 512           # i-window (moving free dim per matmul)
NIW = L // W      # 16 i-windows
WIN = 512         # projection window (rhs free dim)
NWIN = L // WIN   # 8 windows
NQ = NCH // G     # 8 quads of key chunks
SROWS = 32 * (G - 1) + D      # 120 used rows of stacked q/k tiles
VW = 32           # padded vhat chunk width (v | ones | zeros)


def build_graph():
    """Build the single-core Bass graph (SPMD: identical on all 8 cores)."""
    nc = bacc.Bacc(
        "TRN2", target_bir_lowering=False, debug=False, num_devices=8,
        name="attn_dp",
    )

    x_ext = nc.dram_tensor("x", [L, C], F32, kind="ExternalInput").ap()
    # group-stacked projection weights [C, 121]: col 32g+d = w[:, d]
    wqs_ext = nc.dram_tensor("wq_stack", [C, 128], F32,
                             kind="ExternalInput").ap()
    wks_ext = nc.dram_tensor("wk_stack", [C, 128], F32,
                             kind="ExternalInput").ap()
    wv_ext = nc.dram_tensor("wv", [C, D], F32, kind="ExternalInput").ap()
    # wo_stack [128, 193]: rows 32g+d = gamma * w_o[d]; rows 32g+24 col 192 = 1
    wos_ext = nc.dram_tensor("wo_stack", [128, C + 1], F32,
                             kind="ExternalInput").ap()
    out_ext = nc.dram_tensor("out", [L, C], F32, kind="ExternalOutput").ap()

    with tile.TileContext(nc) as tc:
        _build(tc, x_ext, wqs_ext, wks_ext, wv_ext, wos_ext, out_ext)

    nc.compile()
    return nc


def _build(tc, x_ext, wqs_ext, wks_ext, wv_ext, wos_ext, out_ext):
    nc = tc.nc

    with (
        # ---- persistent SBUF ----
        tc.tile_pool(name="const", bufs=1) as const_pool,
        tc.tile_pool(name="xsb", bufs=1) as x_pool,
        tc.tile_pool(name="xbf", bufs=4) as xbf_pool,
        tc.tile_pool(name="xT", bufs=1) as xT_pool,
        tc.tile_pool(name="qkT", bufs=1) as qkT_pool,
        tc.tile_pool(name="vhat", bufs=1) as vhat_pool,
        tc.tile_pool(name="expS", bufs=4) as expS_pool,
        tc.tile_pool(name="pt", bufs=2) as pt_pool,
        tc.tile_pool(name="outst", bufs=4) as outst_pool,
        tc.tile_pool(name="rden", bufs=4) as r_pool,
        # ---- PSUM (8 banks): scores 2x2 + partials 2x1 + epi 2x1 ----
        tc.tile_pool(name="ps_s", bufs=2, space="PSUM") as ps_scores,
        tc.tile_pool(name="ps_acc", bufs=2, space="PSUM") as ps_partials,
        tc.tile_pool(name="ps_e", bufs=2, space="PSUM") as ps_epi,
    ):
        # ================= constants =================
        ident = const_pool.tile([128, 128], BF16)
        make_identity(nc, ident[:])

        # PE warm-up: ~4.5us of dense back-to-back matmuls with no deps so the
        # HAM clock gate un-throttles (1.2 -> 2.4 GHz) before the main pipeline
        warm_rhs = const_pool.tile([128, 256], BF16)
        nc.gpsimd.memset(warm_rhs[:], 0.0)
        warm_ps = ps_epi.tile([128, 256], F32, tag="e")
        for _ in range(18):
            nc.tensor.matmul(warm_ps[:], ident[:], warm_rhs[:],
                             start=True, stop=True)

        # zeros for the partials-bank init matmul (K=1): out = zl.T @ zr = 0
        zl = const_pool.tile([1, 128], BF16)
        zr = const_pool.tile([1, W], BF16)
        nc.gpsimd.memset(zl[:], 0.0)
        nc.gpsimd.memset(zr[:], 0.0)

        wstage = const_pool.tile([128, 760], F32)  # fp32 staging for weights
        def load_weight_bf(ext, rows, cols, stage_off, tag):
            st = wstage[:rows, stage_off:stage_off + cols]
            nc.sync.dma_start(st, ext)
            t = const_pool.tile([rows, cols], BF16, tag=tag)
            nc.vector.tensor_copy(t[:], st)
            return t

        SR = 128
        wqsa = load_weight_bf(wqs_ext[0:128, :], 128, SR, 0, "wqsa")
        wqsb = load_weight_bf(wqs_ext[128:192, :], 64, SR, SR, "wqsb")
        wksa = load_weight_bf(wks_ext[0:128, :], 128, SR, 2 * SR, "wksa")
        wksb = load_weight_bf(wks_ext[128:192, :], 64, SR, 3 * SR, "wksb")
        wva = load_weight_bf(wv_ext[0:128, :], 128, D, 4 * SR, "wva")
        wvb = load_weight_bf(wv_ext[128:192, :], 64, D, 4 * SR + D, "wvb")
        wos = load_weight_bf(wos_ext, 128, C + 1, 4 * SR + 2 * D, "wos")

        # ================= x load + transpose =================
        x_sb = x_pool.tile([128, NCH * C], F32)       # chunk c at cols [C*c, ...)
        xTa = xT_pool.tile([128, L], BF16)            # x^T rows 0..127 (channels)
        xTb = xT_pool.tile([64, L], BF16)             # x^T rows 128..191
        kTs = qkT_pool.tile([128, L], BF16)           # stacked k^T replicas
        qTs = qkT_pool.tile([128, L], BF16)           # stacked q^T replicas
        vhat = vhat_pool.tile([128, NCH * VW], BF16)  # v | ones | zero pad
        nc.gpsimd.memset(vhat[:], 0.0)
        ones_view = vhat.rearrange("p (j d) -> p j d", d=VW)[:, :, D]
        nc.gpsimd.memset(ones_view, 1.0)

        for ch in range(NCH):
            nc.sync.dma_start(x_sb[:, C * ch:C * (ch + 1)],
                              x_ext[128 * ch:128 * (ch + 1), :])

        def transpose_window(w):
            """x^T slab for token window w (4 chunks of 128)."""
            psA = ps_scores.tile([128, WIN], BF16, tag="s", name=f"psA{w}")
            psB = ps_scores.tile([64, WIN], BF16, tag="s", name=f"psB{w}")
            for t in range(4):
                ch = 4 * w + t
                xbf = xbf_pool.tile([128, C], BF16, tag="xbf", name=f"xbf{ch}")
                nc.gpsimd.tensor_copy(xbf[:], x_sb[:, C * ch:C * (ch + 1)])
                nc.tensor.transpose(psA[:, 128 * t:128 * (t + 1)],
                                    xbf[:, 0:128], ident[:])
                nc.tensor.transpose(psB[:, 128 * t:128 * (t + 1)],
                                    xbf[:, 128:192], ident[:])
            nc.vector.tensor_copy(xTa[:, WIN * w:WIN * (w + 1)], psA[:])
            nc.vector.tensor_copy(xTb[:, WIN * w:WIN * (w + 1)], psB[:])

        def project(dst, wa, wb, w, nm):
            ps = ps_epi.tile([128, WIN], F32, tag="e", name=f"pj{nm}{w}")
            sl = slice(WIN * w, WIN * (w + 1))
            nc.tensor.matmul(ps[:], wa[:], xTa[:, sl], start=True, stop=False)
            nc.tensor.matmul(ps[:], wb[:], xTb[:, sl], start=False, stop=True)
            nc.vector.tensor_copy(dst[:, sl], ps[:])

        def vhat_quad(t):
            for j in range(4 * t, 4 * t + 4):
                ps = ps_epi.tile([128, D], F32, tag="e", name=f"pv{j}")
                jsl = slice(128 * j, 128 * (j + 1))
                nc.tensor.matmul(ps[:], xTa[:, jsl], wva[:],
                                 start=True, stop=False)
                nc.tensor.matmul(ps[:], xTb[:, jsl], wvb[:],
                                 start=False, stop=True)
                nc.vector.tensor_copy(vhat[:, VW * j:VW * j + D], ps[:])

        pt_tiles = {}

        def emit_epilogue(piw, s):
            ptb = pt_tiles[piw]
            cidx = (W // 128) * piw + s  # 128-token chunk index
            ep = ps_epi.tile([128, C + 1], F32, tag="e", name=f"ep{piw}_{s}")
            nc.tensor.matmul(ep[:], ptb[:, 128 * s:128 * (s + 1)],
                             wos[:], start=True, stop=True)
            rr = r_pool.tile([128, 1], F32, name=f"rr{piw}_{s}", tag="rr")
            nc.vector.reciprocal(rr[:], ep[:, C:C + 1])
            ot = outst_pool.tile([128, C], F32, name=f"ot{piw}_{s}", tag="ot")
            nc.vector.tensor_scalar_mul(ot[:], ep[:, 0:C], rr[:])
            nc.vector.tensor_add(ot[:], ot[:],
                                 x_sb[:, C * cidx:C * (cidx + 1)])
            nc.sync.dma_start(out_ext[128 * cidx:128 * (cidx + 1), :], ot[:])

        # bulk prologue, window-pipelined emission
        transpose_window(0)
        for w in range(1, NWIN):
            transpose_window(w)
            project(kTs, wksa, wksb, w - 1, "k")
            vhat_quad(w - 1)
        project(kTs, wksa, wksb, NWIN - 1, "k")
        vhat_quad(NWIN - 1)
        project(qTs, wqsa, wqsb, 0, "q")

        # ================= main loop =================
        # i-windows of W=512; key chunks in quads of 4 (row groups 0..3).
        # Each row group's scores land in a distinct PSUM bank (HW rule):
        # groups 0/1 -> scA banks 0/1, groups 2/3 -> scB banks 0/1.
        # The epilogue of window iw-1 is deferred into iw's quad loop so the
        # inter-window dependency chain never stalls the exp stream.
        pending = None
        for iw in range(NIW):
            isl = slice(W * iw, W * (iw + 1))
            partials = ps_partials.tile([128, W], F32, name=f"partials{iw}",
                                        tag="acc")
            # zero-init the bank and set has_written on all 128 partitions so
            # the col-tiled accumulating matmuls below can all use start=False
            nc.tensor.matmul(partials[:, :], zl[:], zr[:],
                             start=True, stop=False, skip_group_check=True)
            for t in range(NQ):
                scs = [ps_scores.tile([128, 1024], F32, tag="s", name=f"sc{iw}_{t}_0"),
                       ps_scores.tile([128, 1024], F32, tag="s", name=f"sc{iw}_{t}_1")]
                for g in range(G):
                    j = G * t + g
                    nc.tensor.matmul(
                        scs[g // 2][:, 512 * (g % 2):512 * (g % 2 + 1)],
                        kTs[32 * g:32 * g + 32, 128 * j:128 * (j + 1)],
                        qTs[32 * g:32 * g + 32, isl],
                        start=True, stop=True,
                        tile_position=(32 * g, 0),
                    )
                ess = []
                for h in range(2):
                    es = expS_pool.tile([128, 1024], BF16, name=f"es{iw}_{t}_{h}", tag="es")
                    nc.scalar.activation(es[:], scs[h][:],
                                         mybir.ActivationFunctionType.Exp)
                    ess.append(es)
                if t == 0 and iw + 1 < NIW:
                    # prefetch next window's q^T early (hides the projection
                    # + copy latency under this window's exp stream)
                    project(qTs, wqsa, wqsb, iw + 1, "q")
                if pending is not None and t < W // 128:
                    emit_epilogue(pending, t)
                for g in (2, 3, 0, 1):
                    j = G * t + g
                    nc.tensor.matmul(
                        partials[32 * g:32 * g + VW, :],
                        vhat[:, VW * j:VW * (j + 1)],
                        ess[g // 2][:, 512 * (g % 2):512 * (g % 2 + 1)],
                        start=False, stop=(t == NQ - 1),
                        tile_position=(0, 32 * g),
                        skip_group_check=True,
                    )

            ptb = pt_pool.tile([128, W], BF16, name=f"ptb{iw}", tag="ptb")
            nc.vector.tensor_copy(ptb[:], partials[:])
            pt_tiles[iw] = ptb
            pending = iw
        for s in range(W // 128):
            emit_epilogue(pending, s)


_CACHE = {}


def _get_graph():
    if "nc" not in _CACHE:
        _CACHE["nc"] = build_graph()
    return _CACHE["nc"]


def make_in_maps(tensor, w_q, w_k, w_v, w_o, gamma):
    x = np.ascontiguousarray(np.asarray(tensor, dtype=np.float32)).reshape(B, L, C)
    wq = np.asarray(w_q, dtype=np.float32)
    wk = np.asarray(w_k, dtype=np.float32)
    wv = np.ascontiguousarray(np.asarray(w_v, dtype=np.float32))
    wo = np.asarray(w_o, dtype=np.float32)

    wq_stack = np.zeros((C, 128), dtype=np.float32)
    wk_stack = np.zeros((C, 128), dtype=np.float32)
    for g in range(G):
        wq_stack[:, 32 * g:32 * g + D] = wq
        wk_stack[:, 32 * g:32 * g + D] = wk

    wo_stack = np.zeros((128, C + 1), dtype=np.float32)
    for g in range(G):
        wo_stack[32 * g:32 * g + D, :C] = wo * np.float32(gamma)
        wo_stack[32 * g + D, C] = 1.0

    return [
        {"x": np.ascontiguousarray(x[b]), "wq_stack": wq_stack,
         "wk_stack": wk_stack, "wv": wv, "wo_stack": wo_stack}
        for b in range(B)
    ]


def kernel(tensor, w_q, w_k, w_v, w_o, gamma):
    nc = _get_graph()
    in_maps = make_in_maps(tensor, w_q, w_k, w_v, w_o, gamma)
    res = run_bass_kernel_spmd(nc, in_maps, core_ids=list(range(B)))
    out = np.stack([np.asarray(res.results[b]["out"]) for b in range(B)])
    return out.reshape(B, 64, 64, C).astype(np.float32)
